# revision 62
# baseline (speedup 1.0000x reference)
"""Trainium2 Bass kernel for distance-based (RBF) attention.

Reference computation (per batch b):
    Q = x @ Wq.T           (N, 64)
    K = x @ Wk.T           (N, 64)
    V = x @ Wv.T           (N, 512)
    dist2[i,j] = |Q_i - K_j|^2
    attn = softmax(-dist2 / (2 lam^2), axis=-1)
    out = attn @ V

Key identity: softmax_j(-(q_i^2 + k_j^2 - 2 q_i.k_j)/(2 lam^2)) ==
softmax_j((q_i.k_j - k_j^2/2) / lam^2) -- the per-row q_i^2 term is a
row-constant and cancels; exp without max-subtraction is safe because the
dominant terms are far inside fp32 range and the normalizer divides the
common scale back out.

Sharding: 8 cores = 4 batches x 2 query-halves. The host supplies x
pre-transposed per core as xt = [512 features, 4096 keys] with the core's
own query half rotated to columns 0:2048 (keys order is a permutation;
softmax is permutation-invariant over keys). No device-side transposes of
x and no cross-core communication.

Engine balance: PE does projections + QK + PV; Act does exp; row-sum
accumulation is SBUF tensor_adds split DVE/Pool (GPSIMD cannot touch PSUM)
folded by tiny ones-matmuls riding the sp psum ring; psum->sbuf eviction
splits DVE/Act; all three DMA-capable queues (SP/Act/Pool) carry transfers.

The -|k|^2/2 softmax bias rides as a 65th contraction row of KT (QT row 64
is ones), so the QK matmul emits q.k - k^2/2 directly and exp needs no
per-partition bias operand. Weight transposes pack 4-8 per psum bank as one
start..stop accumulation group over disjoint columns (one eviction copy
each).
"""

import numpy as np
from contextlib import ExitStack

import concourse.bacc as bacc
import concourse.tile as tile
import concourse.mybir as mybir
from concourse import masks
from concourse.bass_utils import run_bass_kernel_spmd

P = 128
D = 64          # head dim
IN_F = 512
OUT_F = 512
NQ = 2048       # query rows per core
NK = 4096       # keys per core (full batch)
N_CORES = 8
XCH = 8         # column chunks per feature-block DMA of xt
F32 = mybir.dt.float32
F32R = mybir.dt.float32r
AF = mybir.ActivationFunctionType

LAST_RESULTS = None  # test harness reads exec_time_ns from here
_LAST_NC = None
_LAST_IN_MAPS = None


def timed_rerun(n=3):
    """Re-execute the last compiled program; returns list of wall times (s)."""
    import time

    times = []
    for _ in range(n):
        t0 = time.perf_counter()
        run_bass_kernel_spmd(_LAST_NC, _LAST_IN_MAPS, list(range(N_CORES)))
        times.append(time.perf_counter() - t0)
    return times


def build_program(lam: float):
    nc = bacc.Bacc(
        "TRN2", target_bir_lowering=False, debug=False, num_devices=N_CORES
    )
    xt = nc.dram_tensor("xt", [IN_F, NK], F32, kind="ExternalInput").ap()
    wq = nc.dram_tensor("wq", [D, IN_F], F32, kind="ExternalInput").ap()
    wk = nc.dram_tensor("wk", [D, IN_F], F32, kind="ExternalInput").ap()
    wv = nc.dram_tensor("wv", [OUT_F, IN_F], F32, kind="ExternalInput").ap()
    out = nc.dram_tensor("out", [NQ, OUT_F], F32, kind="ExternalOutput").ap()

    inv2 = 1.0 / (lam * lam)
    neghalf = -0.5 * inv2
    CW = NK // XCH  # xt chunk width

    with tile.TileContext(nc) as tc, ExitStack() as octx:
        # ---------- long-lived pools ----------
        cpool = octx.enter_context(tc.tile_pool(name="const", bufs=1))
        ident = cpool.tile([P, P], F32, tag="ident")
        masks.make_identity(nc, ident[:])
        tmp1 = cpool.tile([P, 2], F32, tag="tmp1")
        nc.vector.memset(tmp1[:], 1.0)
        ones2 = cpool.tile([P, 2], F32R, tag="ones2")
        nc.vector.tensor_copy(ones2[:], tmp1[:])
        tmpn = cpool.tile([D, 2], F32, tag="tmpn")
        nc.vector.memset(tmpn[:], -0.5)
        negh64 = cpool.tile([D, 2], F32R, tag="negh64")
        nc.vector.tensor_copy(negh64[:], tmpn[:])

        kt_pool = octx.enter_context(tc.tile_pool(name="kt", bufs=1))
        KT = kt_pool.tile([D + 1, NK], F32R, tag="KT")
        qt_pool = octx.enter_context(tc.tile_pool(name="qt", bufs=1))
        QT = qt_pool.tile([D + 1, NQ], F32R, tag="QT")
        onesq = cpool.tile([1, NQ], F32, tag="onesq")
        nc.vector.memset(onesq[:], 1.0)
        nc.vector.tensor_copy(QT[D : D + 1, :], onesq[:])
        v_pool = octx.enter_context(tc.tile_pool(name="v", bufs=1))
        V = [v_pool.tile([P, OUT_F], F32R, tag=f"V{j}", name=f"V{j}")
             for j in range(32)]

        # ---- phase A: load xt (pre-transposed on host) + project ----
        with ExitStack() as pctx:
            xT_pool = pctx.enter_context(tc.tile_pool(name="xT", bufs=1))
            # XT[fc][ch]: feature rows fc*128..+128, key cols ch*CW..+CW
            XT = [[xT_pool.tile([P, CW], F32R, tag=f"xT{c}_{ch}",
                                name=f"xT{c}_{ch}") for ch in range(XCH)]
                  for c in range(4)]
            wpool = pctx.enter_context(tc.tile_pool(name="w", bufs=1))
            wT_pool = pctx.enter_context(tc.tile_pool(name="wT", bufs=1))
            sq_pool = pctx.enter_context(tc.tile_pool(name="sq", bufs=2))
            tpsum = pctx.enter_context(
                tc.tile_pool(name="tpsum", bufs=3, space="PSUM")
            )
            projpsum = pctx.enter_context(
                tc.tile_pool(name="projpsum", bufs=4, space="PSUM")
            )

            # DMA queues: spread transfers across SP/Act/Pool so no single
            # engine serializes the input stream (DVE/PE cannot DMA)
            dmaq = [nc.sync, nc.scalar, nc.gpsimd]

            def xt_dma(ch):
                qs = dmaq
                for fc in range(4):
                    qs[(ch * 4 + fc) % len(qs)].dma_start(
                        XT[fc][ch][:],
                        xt[fc * P : (fc + 1) * P, ch * CW : (ch + 1) * CW]
                        .bitcast(F32R),
                    )

            # weights first (PE's first work is the weight transposes)
            wq_sb = wpool.tile([D, IN_F], F32, tag="wq_sb")
            nc.sync.dma_start(wq_sb[:], wq)
            wk_sb = wpool.tile([D, IN_F], F32, tag="wk_sb")
            nc.scalar.dma_start(wk_sb[:], wk)
            wv_sb = [wpool.tile([P, IN_F], F32, tag=f"wv_sb{i}", name=f"wv_sb{i}")
                     for i in range(4)]
            for i in range(4):
                dmaq[(2 + i) % 3].dma_start(wv_sb[i][:], wv[i * P : (i + 1) * P, :])
            xt_dma(0)
            xt_dma(1)

            # transpose weights on PE: all 8 wq/wk transposes share one psum
            # bank (disjoint columns, one accumulation group onto zeros),
            # evicted with a single copy; wv packs 4 per bank likewise
            wqkT = wT_pool.tile([P, 512], F32R, tag="wqkT")
            wqT = [wqkT[:, c * D : (c + 1) * D] for c in range(4)]
            wkT = [wqkT[:, 256 + c * D : 256 + (c + 1) * D] for c in range(4)]
            wvT = [wT_pool.tile([P, OUT_F], F32R, tag=f"wvT{c}", name=f"wvT{c}")
                   for c in range(4)]
            tpw = tpsum.tile([P, 512], F32, tag="tpw", bufs=1)
            for c in range(4):
                nc.tensor.matmul(
                    tpw[:, c * D : (c + 1) * D],
                    wq_sb[:, c * P : (c + 1) * P],
                    ident[:D, :D],
                    is_transpose=True,
                    start=(c == 0),
                    stop=False,
                )
            for c in range(4):
                nc.tensor.matmul(
                    tpw[:, 256 + c * D : 256 + (c + 1) * D],
                    wk_sb[:, c * P : (c + 1) * P],
                    ident[:D, :D],
                    is_transpose=True,
                    start=False,
                    stop=(c == 3),
                )
            nc.vector.tensor_copy(wqkT[:], tpw[:])
            for fc in range(4):
                tp = tpsum.tile([P, 512], F32, tag="tpv", bufs=2)
                for oc in range(4):
                    nc.tensor.matmul(
                        tp[:, oc * P : (oc + 1) * P],
                        wv_sb[oc][:, fc * P : (fc + 1) * P],
                        ident[:],
                        is_transpose=True,
                        start=(oc == 0),
                        stop=(oc == 3),
                    )
                nc.vector.tensor_copy(wvT[fc][:], tp[:])

            def xt_cols(fc, col, width):
                """XT view for key cols [col, col+width) in feature block fc
                (must stay within one chunk)."""
                ch, off = divmod(col, CW)
                assert off + width <= CW
                return XT[fc][ch][:, off : off + width]

            # per-chunk pipeline: next chunk's DMAs, then KT/QT blocks, V
            # blocks, and biask for this chunk's keys.  biask[jc][j,0] =
            # -|K_j|^2 / (2 lam^2), per 128-key chunk.
            BPC = CW // 512  # 512-col projection blocks per chunk
            for ch in range(XCH):
                if ch + 2 < XCH:
                    xt_dma(ch + 2)
                for b in range(BPC):
                    nb = ch * BPC + b
                    col = nb * 512
                    pp = projpsum.tile([D, 512], F32, tag="pp", name="pp")
                    for fc in range(4):
                        nc.tensor.matmul(
                            pp[:],
                            wkT[fc],
                            xt_cols(fc, col, 512),
                            start=(fc == 0),
                            stop=(fc == 3),
                        )
                    nc.vector.tensor_copy(KT[:D, col : col + 512], pp[:])
                    if col < NQ:
                        pp = projpsum.tile([D, 512], F32, tag="pp", name="pp")
                        for fc in range(4):
                            nc.tensor.matmul(
                                pp[:],
                                wqT[fc],
                                xt_cols(fc, col, 512),
                                start=(fc == 0),
                                stop=(fc == 3),
                            )
                        nc.vector.tensor_copy(QT[:D, col : col + 512], pp[:])
                for jc in range(ch * (32 // XCH), (ch + 1) * (32 // XCH)):
                    pv = projpsum.tile([P, OUT_F], F32, tag="pp", name="pv")
                    for fc in range(4):
                        nc.tensor.matmul(
                            pv[:],
                            xt_cols(fc, jc * P, P),
                            wvT[fc][:],
                            start=(fc == 0),
                            stop=(fc == 3),
                        )
                    if jc % 2 == 0:
                        nc.vector.tensor_copy(V[jc][:], pv[:])
                    else:
                        nc.scalar.copy(V[jc][:], pv[:])
                for b in range(BPC):
                    nb = ch * BPC + b
                    col = nb * 512
                    sq = sq_pool.tile([D, 512], F32R, tag="sq", bufs=2)
                    nc.gpsimd.tensor_mul(
                        sq[:], KT[:D, col : col + 512], KT[:D, col : col + 512]
                    )
                    kp = projpsum.tile([2, 512], F32, tag="pp", name="kp")
                    nc.tensor.matmul(kp[:], negh64[:], sq[:], start=True, stop=True)
                    nc.vector.tensor_copy(KT[D : D + 1, col : col + 512], kp[0:1, :])

        # ---------- phase C: attention ----------
        with ExitStack() as actx:
            opsum = actx.enter_context(
                tc.tile_pool(name="opsum", bufs=1, space="PSUM")
            )
            spsum = actx.enter_context(
                tc.tile_pool(name="spsum", bufs=4, space="PSUM")
            )
            ptpool = actx.enter_context(tc.tile_pool(name="pt", bufs=6))
            onpool = actx.enter_context(tc.tile_pool(name="on", bufs=4))
            recpool = actx.enter_context(tc.tile_pool(name="rec", bufs=4))


            def qk_exp(ib, jc):
                sp = spsum.tile([P, 512], F32, tag="sp")
                nc.tensor.matmul(
                    sp[:],
                    KT[:, jc * P : (jc + 1) * P],
                    QT[:, ib * 512 : (ib + 1) * 512],
                    start=True,
                    stop=True,
                )
                pt = ptpool.tile([P, 512], F32R, tag="pt")
                nc.scalar.activation(pt[:], sp[:], AF.Exp, scale=inv2)
                return pt

            # software pipeline: QK+exp for step s+1 are emitted before the
            # PV/row-sum group of step s so exp latency hides under PV.
            # Row sums accumulate in SBUF on DVE (even jc) and Pool (odd jc)
            # -- concurrent psum accumulators would collide in one 2KB zero
            # region -- then fold via tiny start&stop ones-matmuls per ib.
            steps = [(ib, jc) for ib in range(NQ // 512) for jc in range(32)]
            outp = None
            lsD = lsP = None
            pts = [qk_exp(*steps[0]), qk_exp(*steps[1])]
            for s, (ib, jc) in enumerate(steps):
                if jc == 0:
                    outp = [opsum.tile([P, OUT_F], F32, tag=f"op{i}",
                                       name=f"op{ib}_{i}")
                            for i in range(4)]
                if s + 2 < len(steps):
                    pts.append(qk_exp(*steps[s + 2]))
                pt_cur = pts.pop(0)
                for ic in range(4):
                    nc.tensor.matmul(
                        outp[ic][:],
                        pt_cur[:, ic * P : (ic + 1) * P],
                        V[jc][:],
                        start=(jc == 0),
                        stop=(jc == 31),
                    )
                # row sums: DVE owns even jc (and jc 30's pre-fold + jc 31),
                # Pool owns odd jc up to 29; keeps the last-step chain short
                if jc == 0:
                    lsD = ptpool.tile([P, 512], F32R, tag="lsD",
                                      name=f"lsD{ib}", bufs=2)
                    nc.vector.tensor_copy(lsD[:], pt_cur[:])
                elif jc == 1:
                    lsP = ptpool.tile([P, 512], F32R, tag="lsP",
                                      name=f"lsP{ib}", bufs=2)
                    nc.gpsimd.tensor_copy(lsP[:], pt_cur[:])
                elif jc % 2 == 0 and jc != 30:
                    nc.vector.tensor_add(lsD[:], lsD[:], pt_cur[:])
                elif jc % 2 == 1 and jc != 31:
                    nc.gpsimd.tensor_add(lsP[:], lsP[:], pt_cur[:])
                if jc == 29:
                    # claim lp's ring slot before the boundary crunch
                    lp = spsum.tile([P, 8], F32, tag="sp", name=f"lp{ib}")
                if jc == 30:
                    # pre-fold including pt30 so only pt31 is outstanding
                    nc.vector.tensor_add(lsD[:], lsD[:], pt_cur[:])
                    nc.vector.tensor_add(lsD[:], lsD[:], lsP[:])
                if jc != 31:
                    continue
                # lp's ones-matmuls accumulate the pre-folded lsD plus pt31
                # directly, keeping the slow DVE add off the tail chain
                for ic in range(4):
                    nc.tensor.matmul(
                        lp[:, 2 * ic : 2 * ic + 2],
                        lsD[:, ic * P : (ic + 1) * P],
                        ones2[:],
                        start=True,
                        stop=False,
                    )
                    nc.tensor.matmul(
                        lp[:, 2 * ic : 2 * ic + 2],
                        pt_cur[:, ic * P : (ic + 1) * P],
                        ones2[:],
                        start=False,
                        stop=True,
                    )
                recs = []
                for ic in range(4):
                    rec = recpool.tile([P, 2], F32, tag="rec")
                    nc.vector.reciprocal(rec[:], lp[:, 2 * ic : 2 * ic + 2])
                    recs.append(rec)
                for ic in range(4):
                    rec = recs[ic]
                    on = onpool.tile([P, OUT_F], F32, tag="on")
                    r0 = ib * 512 + ic * P
                    dq = nc.sync if ic % 2 == 0 else nc.gpsimd
                    if ib == 3:
                        # tail: halve the normalize->DMA chain by columns so
                        # the final store starts ~700ns earlier
                        for hf in range(2):
                            cols = slice(hf * 256, (hf + 1) * 256)
                            if ic % 2 == 0:
                                nc.vector.tensor_scalar_mul(
                                    on[:, cols], outp[ic][:, cols], rec[:, 0:1]
                                )
                            else:
                                nc.scalar.mul(
                                    on[:, cols], outp[ic][:, cols], rec[:, 0:1]
                                )
                            dq.dma_start(out[r0 : r0 + P, cols], on[:, cols])
                    else:
                        nc.vector.tensor_scalar_mul(
                            on[:], outp[ic][:], rec[:, 0:1]
                        )
                        dq.dma_start(out[r0 : r0 + P, :], on[:])

    nc.compile()
    return nc


_CACHE = {}


def _get_program(lam: float):
    key = round(float(lam), 9)
    if key not in _CACHE:
        _CACHE[key] = build_program(key)
    return _CACHE[key]


def kernel(x, Wq, Wk, Wv, log_lambda):
    x = np.asarray(x, dtype=np.float32)
    Wq = np.ascontiguousarray(np.asarray(Wq, dtype=np.float32))
    Wk = np.ascontiguousarray(np.asarray(Wk, dtype=np.float32))
    Wv = np.ascontiguousarray(np.asarray(Wv, dtype=np.float32))
    lam = float(np.clip(np.exp(np.asarray(log_lambda, np.float32)[0]), 1e-3, None))

    nc = _get_program(lam)

    in_maps = []
    for c in range(N_CORES):
        b, h = divmod(c, 2)
        xT = x[b].T  # [512, 4096] view
        if h == 0:
            xt_ = np.ascontiguousarray(xT)
        else:
            xt_ = np.ascontiguousarray(
                np.concatenate([xT[:, NQ:], xT[:, :NQ]], axis=1)
            )
        in_maps.append({"xt": xt_, "wq": Wq, "wk": Wk, "wv": Wv})

    res = run_bass_kernel_spmd(nc, in_maps, list(range(N_CORES)))
    global LAST_RESULTS, _LAST_NC, _LAST_IN_MAPS
    LAST_RESULTS = res
    _LAST_NC = nc
    _LAST_IN_MAPS = in_maps

    out = np.empty((4, 2 * NQ, OUT_F), np.float32)
    for c in range(N_CORES):
        b, h = divmod(c, 2)
        out[b, h * NQ : (h + 1) * NQ] = res.results[c]["out"]
    return out


# revision 66
# speedup vs baseline: 1.0004x; 1.0004x over previous
"""Trainium2 Bass kernel for distance-based (RBF) attention.

Reference computation (per batch b):
    Q = x @ Wq.T           (N, 64)
    K = x @ Wk.T           (N, 64)
    V = x @ Wv.T           (N, 512)
    dist2[i,j] = |Q_i - K_j|^2
    attn = softmax(-dist2 / (2 lam^2), axis=-1)
    out = attn @ V

Key identity: softmax_j(-(q_i^2 + k_j^2 - 2 q_i.k_j)/(2 lam^2)) ==
softmax_j((q_i.k_j - k_j^2/2) / lam^2) -- the per-row q_i^2 term is a
row-constant and cancels; exp without max-subtraction is safe because the
dominant terms are far inside fp32 range and the normalizer divides the
common scale back out.

Sharding: 8 cores = 4 batches x 2 query-halves. The host supplies x
pre-transposed per core as xt = [512 features, 4096 keys] with the core's
own query half rotated to columns 0:2048 (keys order is a permutation;
softmax is permutation-invariant over keys). No device-side transposes of
x and no cross-core communication.

Engine balance: PE does projections + QK + PV; Act does exp; row-sum
accumulation is SBUF tensor_adds split DVE/Pool (GPSIMD cannot touch PSUM)
folded by tiny ones-matmuls riding the sp psum ring; psum->sbuf eviction
splits DVE/Act; all three DMA-capable queues (SP/Act/Pool) carry transfers.

The -|k|^2/2 softmax bias rides as a 65th contraction row of KT (QT row 64
is ones), so the QK matmul emits q.k - k^2/2 directly and exp needs no
per-partition bias operand. Weight transposes pack 4-8 per psum bank as one
start..stop accumulation group over disjoint columns (one eviction copy
each).
"""

import numpy as np
from contextlib import ExitStack

import concourse.bacc as bacc
import concourse.tile as tile
import concourse.mybir as mybir
from concourse import masks
from concourse.bass_utils import run_bass_kernel_spmd

P = 128
D = 64          # head dim
IN_F = 512
OUT_F = 512
NQ = 2048       # query rows per core
NK = 4096       # keys per core (full batch)
N_CORES = 8
XCH = 8         # column chunks per feature-block DMA of xt
F32 = mybir.dt.float32
F32R = mybir.dt.float32r
AF = mybir.ActivationFunctionType

LAST_RESULTS = None  # test harness reads exec_time_ns from here
_LAST_NC = None
_LAST_IN_MAPS = None


def timed_rerun(n=3):
    """Re-execute the last compiled program; returns list of wall times (s)."""
    import time

    times = []
    for _ in range(n):
        t0 = time.perf_counter()
        run_bass_kernel_spmd(_LAST_NC, _LAST_IN_MAPS, list(range(N_CORES)))
        times.append(time.perf_counter() - t0)
    return times


def build_program(lam: float):
    nc = bacc.Bacc(
        "TRN2", target_bir_lowering=False, debug=False, num_devices=N_CORES
    )
    xt = nc.dram_tensor("xt", [IN_F, NK], F32, kind="ExternalInput").ap()
    wq = nc.dram_tensor("wq", [D, IN_F], F32, kind="ExternalInput").ap()
    wk = nc.dram_tensor("wk", [D, IN_F], F32, kind="ExternalInput").ap()
    wv = nc.dram_tensor("wv", [OUT_F, IN_F], F32, kind="ExternalInput").ap()
    out = nc.dram_tensor("out", [NQ, OUT_F], F32, kind="ExternalOutput").ap()

    inv2 = 1.0 / (lam * lam)
    neghalf = -0.5 * inv2
    CW = NK // XCH  # xt chunk width

    with tile.TileContext(nc) as tc, ExitStack() as octx:
        # ---------- long-lived pools ----------
        cpool = octx.enter_context(tc.tile_pool(name="const", bufs=1))
        ident = cpool.tile([P, P], F32, tag="ident")
        masks.make_identity(nc, ident[:])
        tmp1 = cpool.tile([P, 2], F32, tag="tmp1")
        nc.vector.memset(tmp1[:], 1.0)
        ones2 = cpool.tile([P, 2], F32R, tag="ones2")
        nc.vector.tensor_copy(ones2[:], tmp1[:])
        tmpn = cpool.tile([D, 2], F32, tag="tmpn")
        nc.vector.memset(tmpn[:], -0.5)
        negh64 = cpool.tile([D, 2], F32R, tag="negh64")
        nc.vector.tensor_copy(negh64[:], tmpn[:])

        kt_pool = octx.enter_context(tc.tile_pool(name="kt", bufs=1))
        KT = kt_pool.tile([D + 1, NK], F32R, tag="KT")
        qt_pool = octx.enter_context(tc.tile_pool(name="qt", bufs=1))
        QT = qt_pool.tile([D + 1, NQ], F32R, tag="QT")
        onesq = cpool.tile([1, NQ], F32, tag="onesq")
        nc.vector.memset(onesq[:], 1.0)
        nc.vector.tensor_copy(QT[D : D + 1, :], onesq[:])
        v_pool = octx.enter_context(tc.tile_pool(name="v", bufs=1))
        V = [v_pool.tile([P, OUT_F], F32R, tag=f"V{j}", name=f"V{j}")
             for j in range(32)]

        # ---- phase A: load xt (pre-transposed on host) + project ----
        with ExitStack() as pctx:
            xT_pool = pctx.enter_context(tc.tile_pool(name="xT", bufs=1))
            # XT[fc][ch]: feature rows fc*128..+128, key cols ch*CW..+CW
            XT = [[xT_pool.tile([P, CW], F32R, tag=f"xT{c}_{ch}",
                                name=f"xT{c}_{ch}") for ch in range(XCH)]
                  for c in range(4)]
            wpool = pctx.enter_context(tc.tile_pool(name="w", bufs=1))
            wT_pool = pctx.enter_context(tc.tile_pool(name="wT", bufs=1))
            sq_pool = pctx.enter_context(tc.tile_pool(name="sq", bufs=2))
            tpsum = pctx.enter_context(
                tc.tile_pool(name="tpsum", bufs=3, space="PSUM")
            )
            projpsum = pctx.enter_context(
                tc.tile_pool(name="projpsum", bufs=4, space="PSUM")
            )

            # DMA queues: spread transfers across SP/Act/Pool so no single
            # engine serializes the input stream (DVE/PE cannot DMA)
            dmaq = [nc.sync, nc.scalar, nc.gpsimd]

            def xt_dma(ch):
                qs = dmaq
                for fc in range(4):
                    qs[(ch * 4 + fc) % len(qs)].dma_start(
                        XT[fc][ch][:],
                        xt[fc * P : (fc + 1) * P, ch * CW : (ch + 1) * CW]
                        .bitcast(F32R),
                    )

            # weights first (PE's first work is the weight transposes)
            wq_sb = wpool.tile([D, IN_F], F32, tag="wq_sb")
            nc.sync.dma_start(wq_sb[:], wq)
            wk_sb = wpool.tile([D, IN_F], F32, tag="wk_sb")
            nc.scalar.dma_start(wk_sb[:], wk)
            wv_sb = [wpool.tile([P, IN_F], F32, tag=f"wv_sb{i}", name=f"wv_sb{i}")
                     for i in range(4)]
            for i in range(4):
                dmaq[(2 + i) % 3].dma_start(wv_sb[i][:], wv[i * P : (i + 1) * P, :])
            xt_dma(0)
            xt_dma(1)

            # transpose weights on PE: all 8 wq/wk transposes share one psum
            # bank (disjoint columns, one accumulation group onto zeros),
            # evicted with a single copy; wv packs 4 per bank likewise
            wqkT = wT_pool.tile([P, 512], F32R, tag="wqkT")
            wqT = [wqkT[:, c * D : (c + 1) * D] for c in range(4)]
            wkT = [wqkT[:, 256 + c * D : 256 + (c + 1) * D] for c in range(4)]
            wvT = [wT_pool.tile([P, OUT_F], F32R, tag=f"wvT{c}", name=f"wvT{c}")
                   for c in range(4)]
            tpw = tpsum.tile([P, 512], F32, tag="tpw", bufs=1)
            for c in range(4):
                nc.tensor.matmul(
                    tpw[:, c * D : (c + 1) * D],
                    wq_sb[:, c * P : (c + 1) * P],
                    ident[:D, :D],
                    is_transpose=True,
                    start=(c == 0),
                    stop=False,
                )
            for c in range(4):
                nc.tensor.matmul(
                    tpw[:, 256 + c * D : 256 + (c + 1) * D],
                    wk_sb[:, c * P : (c + 1) * P],
                    ident[:D, :D],
                    is_transpose=True,
                    start=False,
                    stop=(c == 3),
                )
            nc.vector.tensor_copy(wqkT[:], tpw[:])
            for fc in range(4):
                tp = tpsum.tile([P, 512], F32, tag="tpv", bufs=2)
                for oc in range(4):
                    nc.tensor.matmul(
                        tp[:, oc * P : (oc + 1) * P],
                        wv_sb[oc][:, fc * P : (fc + 1) * P],
                        ident[:],
                        is_transpose=True,
                        start=(oc == 0),
                        stop=(oc == 3),
                    )
                nc.vector.tensor_copy(wvT[fc][:], tp[:])

            def xt_cols(fc, col, width):
                """XT view for key cols [col, col+width) in feature block fc
                (must stay within one chunk)."""
                ch, off = divmod(col, CW)
                assert off + width <= CW
                return XT[fc][ch][:, off : off + width]

            # per-chunk pipeline: next chunk's DMAs, then KT/QT blocks, V
            # blocks, and biask for this chunk's keys.  biask[jc][j,0] =
            # -|K_j|^2 / (2 lam^2), per 128-key chunk.
            BPC = CW // 512  # 512-col projection blocks per chunk
            for ch in range(XCH):
                if ch + 2 < XCH:
                    xt_dma(ch + 2)
                for b in range(BPC):
                    nb = ch * BPC + b
                    col = nb * 512
                    pp = projpsum.tile([D, 512], F32, tag="pp", name="pp")
                    for fc in range(4):
                        nc.tensor.matmul(
                            pp[:],
                            wkT[fc],
                            xt_cols(fc, col, 512),
                            start=(fc == 0),
                            stop=(fc == 3),
                        )
                    nc.vector.tensor_copy(KT[:D, col : col + 512], pp[:])
                    if col < NQ:
                        pp = projpsum.tile([D, 512], F32, tag="pp", name="pp")
                        for fc in range(4):
                            nc.tensor.matmul(
                                pp[:],
                                wqT[fc],
                                xt_cols(fc, col, 512),
                                start=(fc == 0),
                                stop=(fc == 3),
                            )
                        nc.vector.tensor_copy(QT[:D, col : col + 512], pp[:])
                for jc in range(ch * (32 // XCH), (ch + 1) * (32 // XCH)):
                    pv = projpsum.tile([P, OUT_F], F32, tag="pp", name="pv")
                    for fc in range(4):
                        nc.tensor.matmul(
                            pv[:],
                            xt_cols(fc, jc * P, P),
                            wvT[fc][:],
                            start=(fc == 0),
                            stop=(fc == 3),
                        )
                    if jc % 2 == 0:
                        nc.vector.tensor_copy(V[jc][:], pv[:])
                    else:
                        nc.scalar.copy(V[jc][:], pv[:])
                for b in range(BPC):
                    nb = ch * BPC + b
                    col = nb * 512
                    sq = sq_pool.tile([D, 512], F32R, tag="sq", bufs=2)
                    nc.gpsimd.tensor_mul(
                        sq[:], KT[:D, col : col + 512], KT[:D, col : col + 512]
                    )
                    kp = projpsum.tile([2, 512], F32, tag="pp", name="kp")
                    nc.tensor.matmul(kp[:], negh64[:], sq[:], start=True, stop=True)
                    nc.vector.tensor_copy(KT[D : D + 1, col : col + 512], kp[0:1, :])

        # ---------- phase C: attention ----------
        with ExitStack() as actx:
            opsum = actx.enter_context(
                tc.tile_pool(name="opsum", bufs=1, space="PSUM")
            )
            spsum = actx.enter_context(
                tc.tile_pool(name="spsum", bufs=4, space="PSUM")
            )
            ptpool = actx.enter_context(tc.tile_pool(name="pt", bufs=6))
            onpool = actx.enter_context(tc.tile_pool(name="on", bufs=4))
            recpool = actx.enter_context(tc.tile_pool(name="rec", bufs=4))


            def qk_exp(ib, jc):
                sp = spsum.tile([P, 512], F32, tag="sp")
                nc.tensor.matmul(
                    sp[:],
                    KT[:, jc * P : (jc + 1) * P],
                    QT[:, ib * 512 : (ib + 1) * 512],
                    start=True,
                    stop=True,
                )
                pt = ptpool.tile([P, 512], F32R, tag="pt")
                nc.scalar.activation(pt[:], sp[:], AF.Exp, scale=inv2)
                return pt

            # software pipeline: QK+exp for step s+1 are emitted before the
            # PV/row-sum group of step s so exp latency hides under PV.
            # Row sums accumulate in SBUF on DVE (even jc) and Pool (odd jc)
            # -- concurrent psum accumulators would collide in one 2KB zero
            # region -- then fold via tiny start&stop ones-matmuls per ib.
            steps = [(ib, jc) for ib in range(NQ // 512) for jc in range(32)]
            outp = None
            lsD = lsP = None
            pts = [qk_exp(*steps[0]), qk_exp(*steps[1])]
            for s, (ib, jc) in enumerate(steps):
                if jc == 0:
                    outp = [opsum.tile([P, OUT_F], F32, tag=f"op{i}",
                                       name=f"op{ib}_{i}")
                            for i in range(4)]
                if s + 2 < len(steps):
                    pts.append(qk_exp(*steps[s + 2]))
                pt_cur = pts.pop(0)
                for ic in range(4):
                    nc.tensor.matmul(
                        outp[ic][:],
                        pt_cur[:, ic * P : (ic + 1) * P],
                        V[jc][:],
                        start=(jc == 0),
                        stop=(jc == 31),
                    )
                # row sums: DVE owns even jc (and jc 30's pre-fold + jc 31),
                # Pool owns odd jc up to 29; keeps the last-step chain short
                if jc == 0:
                    lsD = ptpool.tile([P, 512], F32R, tag="lsD",
                                      name=f"lsD{ib}", bufs=2)
                    nc.vector.tensor_copy(lsD[:], pt_cur[:])
                elif jc == 1:
                    lsP = ptpool.tile([P, 512], F32R, tag="lsP",
                                      name=f"lsP{ib}", bufs=2)
                    nc.gpsimd.tensor_copy(lsP[:], pt_cur[:])
                elif jc % 2 == 0 and jc != 30:
                    nc.vector.tensor_add(lsD[:], lsD[:], pt_cur[:])
                elif jc % 2 == 1 and jc != 31:
                    nc.gpsimd.tensor_add(lsP[:], lsP[:], pt_cur[:])
                if jc == 29:
                    # claim lp's ring slot before the boundary crunch
                    lp = spsum.tile([P, 8], F32, tag="sp", name=f"lp{ib}")
                if jc == 30:
                    # pre-fold including pt30 so only pt31 is outstanding
                    nc.vector.tensor_add(lsD[:], lsD[:], pt_cur[:])
                    nc.vector.tensor_add(lsD[:], lsD[:], lsP[:])
                if jc != 31:
                    continue
                # lp's ones-matmuls accumulate the pre-folded lsD plus pt31
                # directly, keeping the slow DVE add off the tail chain
                for ic in range(4):
                    nc.tensor.matmul(
                        lp[:, 2 * ic : 2 * ic + 2],
                        lsD[:, ic * P : (ic + 1) * P],
                        ones2[:],
                        start=True,
                        stop=False,
                    )
                    nc.tensor.matmul(
                        lp[:, 2 * ic : 2 * ic + 2],
                        pt_cur[:, ic * P : (ic + 1) * P],
                        ones2[:],
                        start=False,
                        stop=True,
                    )
                recs = []
                for ic in range(4):
                    rec = recpool.tile([P, 2], F32, tag="rec")
                    nc.vector.reciprocal(rec[:], lp[:, 2 * ic : 2 * ic + 2])
                    recs.append(rec)
                for ic in range(4):
                    rec = recs[ic]
                    on = onpool.tile([P, OUT_F], F32, tag="on")
                    r0 = ib * 512 + ic * P
                    dq = nc.sync if ic % 2 == 0 else nc.gpsimd
                    if ib == 3:
                        # tail: halve the normalize->DMA chain by columns so
                        # the final store starts ~700ns earlier
                        for hf in range(2):
                            cols = slice(hf * 256, (hf + 1) * 256)
                            if ic % 2 == 0:
                                nc.vector.tensor_scalar_mul(
                                    on[:, cols], outp[ic][:, cols], rec[:, 0:1]
                                )
                            else:
                                nc.scalar.mul(
                                    on[:, cols], outp[ic][:, cols], rec[:, 0:1]
                                )
                            dq.dma_start(out[r0 : r0 + P, cols], on[:, cols])
                    else:
                        nc.vector.tensor_scalar_mul(
                            on[:], outp[ic][:], rec[:, 0:1]
                        )
                        dq.dma_start(out[r0 : r0 + P, :], on[:])

    nc.compile()
    return nc


_CACHE = {}


def _get_program(lam: float):
    key = round(float(lam), 9)
    if key not in _CACHE:
        _CACHE[key] = build_program(key)
    return _CACHE[key]


def kernel(x, Wq, Wk, Wv, log_lambda):
    x = np.asarray(x, dtype=np.float32)
    Wq = np.ascontiguousarray(np.asarray(Wq, dtype=np.float32))
    Wk = np.ascontiguousarray(np.asarray(Wk, dtype=np.float32))
    Wv = np.ascontiguousarray(np.asarray(Wv, dtype=np.float32))
    lam = float(np.clip(np.exp(np.asarray(log_lambda, np.float32)[0]), 1e-3, None))

    nc = _get_program(lam)

    in_maps = []
    for c in range(N_CORES):
        b, h = divmod(c, 2)
        xT = x[b].T  # [512, 4096] view
        if h == 0:
            xt_ = np.ascontiguousarray(xT)
        else:
            xt_ = np.ascontiguousarray(
                np.concatenate([xT[:, NQ:], xT[:, :NQ]], axis=1)
            )
        in_maps.append({"xt": xt_, "wq": Wq, "wk": Wk, "wv": Wv})

    res = run_bass_kernel_spmd(nc, in_maps, list(range(N_CORES)))
    global LAST_RESULTS, _LAST_NC, _LAST_IN_MAPS
    LAST_RESULTS = res
    _LAST_NC = nc
    _LAST_IN_MAPS = in_maps

    out = np.empty((4, 2 * NQ, OUT_F), np.float32)
    for c in range(N_CORES):
        b, h = divmod(c, 2)
        out[b, h * NQ : (h + 1) * NQ] = res.results[c]["out"]
    return out
